# revision 9
# baseline (speedup 1.0000x reference)
"""CenterLoss kernel for Trainium2 (raw Bass/Bacc, no Tile), 8-core
data-parallel.

Key algebraic insight: the reference builds the full [B, C] squared-
distance matrix and masks it with one-hot(labels), so only
distmat[i, labels[i]] survives.  The loss is therefore

    loss = (1/B) * sum_i || x_i - centers[labels[i]] ||^2

which needs only a gather of each sample's center row (indirect DMA), not
the [4096, 10000] matmul.

Sharding: data-parallel over the batch.  Each of the 8 cores gets 512
samples (x shard + labels shard) and the full replicated centers table in
DRAM; it gathers its 512 center rows, computes per-partition partial sums
of ||x - c||^2 on device, and the host reduces the 8x[128,2] partials.

v2 vs v1: the dominant cost in v1 was SWDGE descriptor generation on the
GpSimd Q7 core — 4 indirect gathers x ~1.4us apiece (the ~1us fixed
overhead per SWDGE instruction dwarfs the 0.34ns/descriptor part).  v1
also burned ~1.3us of tail on a PE partition-reduction + PSUM copy.
v2 issues TWO indirect gathers (256 rows each, offset AP [128,2]) so Q7
issue time drops ~5.5us -> ~2.2us while still double-buffering the
compute against the second gather's flight time; the tail writes raw
per-partition accumulator columns ([128,2] f32) straight to DRAM from the
Scalar engine and the host does the final 256-element sum.

Per core layout (interleaved: partition p, column-chunk a hold sample
4p + a, a in 0..3; halves A = chunks {0,1}, B = chunks {2,3}):
  Sync   : labels DMA ([128,4] int32), then x as two [128,1024] DMAs with
           4 KB contiguous per-partition strips
  GpSimd : 2 indirect gathers (SWDGE), offset AP = lab_sb[:, 2h:2h+2]
  Vector : subtract half A; subtract half B; then (d*d, accum add) via
           tensor_tensor_reduce for half B -> partials[:,1]
  Scalar : Square activation w/ accum over half A -> partials[:,0]; final
           out DMA (HWDGE on ACT) of partials [128,2]
Host: sum(partials) / BATCH, summed over the 8 cores.

Manual semaphores; no Tile exit drain+butterfly+sem-clear (the bass entry
preamble clears sems, so re-execution stays safe).
"""

from contextlib import ExitStack

import numpy as np

import concourse.bacc as bacc
import concourse.bass as bass
from concourse import mybir
from concourse.bass_utils import run_bass_kernel_spmd

BATCH = 4096
NUM_CLASSES = 10000
FEAT_DIM = 512
N_CORES = 8
BPC = BATCH // N_CORES   # samples per core = 512
P = 128                  # SBUF partitions
CHUNKS = BPC // P        # 4 column-chunks of 128 samples per core
HALF = CHUNKS // 2       # 2 chunks per gather half
HF = HALF * FEAT_DIM     # free-dim elements per half = 1024

AF = mybir.AluOpType

_NC_CACHE = {}


def _build_bass():
    nc = bacc.Bacc(None, target_bir_lowering=False)

    x_in = nc.dram_tensor("x", [BPC, FEAT_DIM], mybir.dt.float32,
                          kind="ExternalInput")
    lab_in = nc.dram_tensor("labels", [BPC], mybir.dt.int32,
                            kind="ExternalInput")
    cen_in = nc.dram_tensor("centers", [NUM_CLASSES, FEAT_DIM],
                            mybir.dt.float32, kind="ExternalInput")
    out_t = nc.dram_tensor("out", [P, 2], mybir.dt.float32,
                           kind="ExternalOutput")

    with ExitStack() as ctx:
        ec = ctx.enter_context
        lab_sb = ec(nc.sbuf_tensor("lab_sb", [P, CHUNKS], mybir.dt.int32))
        xt = ec(nc.sbuf_tensor("xt", [P, CHUNKS * FEAT_DIM],
                               mybir.dt.float32))
        ct = ec(nc.sbuf_tensor("ct", [P, CHUNKS * FEAT_DIM],
                               mybir.dt.float32))
        dd = ec(nc.sbuf_tensor("dd", [P, CHUNKS * FEAT_DIM],
                               mybir.dt.float32))
        sq = ec(nc.sbuf_tensor("sq", [P, CHUNKS * FEAT_DIM],
                               mybir.dt.float32))
        partials = ec(nc.sbuf_tensor("partials", [P, 2], mybir.dt.float32))
        s_lab = ec(nc.semaphore("s_lab"))
        s_xs = [ec(nc.semaphore(f"s_x{h}")) for h in range(2)]
        s_cts = [ec(nc.semaphore(f"s_ct{h}")) for h in range(2)]
        s_sub = ec(nc.semaphore("s_sub"))
        s_sq = ec(nc.semaphore("s_sq"))
        s_ttr = ec(nc.semaphore("s_ttr"))
        s_out = ec(nc.semaphore("s_out"))

        # ---- Sync: labels first (gathers depend on them), then x as two
        # halves with 4 KB contiguous per-partition strips (partition p holds
        # rows 4p..4p+3; half h covers chunks {2h, 2h+1} = rows 4p+2h+g).
        nc.sync.dma_start(
            out=lab_sb[:],
            in_=lab_in[:].rearrange("(p a) -> p a", a=CHUNKS),
        ).then_inc(s_lab, 16)
        for h in range(2):
            nc.sync.dma_start(
                out=xt[:, h * HF:(h + 1) * HF],
                in_=x_in[:].rearrange(
                    "(p h g) f -> p h (g f)", h=2, g=HALF)[:, h, :],
            ).then_inc(s_xs[h], 16)

        # ---- GpSimd: two 256-row gathers (SWDGE).  Offset AP [128,2]:
        # ct[p, (2h+g)*F : ...] = centers[lab_sb[p, 2h+g]] — one SWDGE
        # instruction per half amortizes the ~1us fixed issue cost. ----
        nc.gpsimd.wait_ge(s_lab, 16)
        for h in range(2):
            nc.gpsimd.indirect_dma_start(
                out=ct[:, h * HF:(h + 1) * HF],
                out_offset=None,
                in_=cen_in[:],
                in_offset=bass.IndirectOffsetOnAxis(
                    ap=lab_sb[:, 2 * h:2 * h + 2], axis=0),
            ).then_inc(s_cts[h], 16)

        # ---- Vector: subtract per half; half B also gets its square+reduce
        # here (frees Scalar to finish half A and issue the out DMA). ----
        for h in range(2):
            sl = slice(h * HF, (h + 1) * HF)
            nc.vector.wait_ge(s_xs[h], 16)
            nc.vector.wait_ge(s_cts[h], 16)
            nc.vector.tensor_tensor(
                out=dd[:, sl], in0=xt[:, sl], in1=ct[:, sl],
                op=AF.subtract).then_inc(s_sub, 1)
        # ---- Scalar: per-half square + free-dim accumulate ----
        for h in range(2):
            sl = slice(h * HF, (h + 1) * HF)
            nc.scalar.wait_ge(s_sub, h + 1)
            nc.scalar.activation(
                out=sq[:, sl], in_=dd[:, sl],
                func=mybir.ActivationFunctionType.Square,
                accum_out=partials[:, h:h + 1]).then_inc(s_sq, 1)

        # ---- Scalar: output DMA (HWDGE on ACT) ----
        nc.scalar.wait_ge(s_sq, 2)
        nc.scalar.dma_start(out=out_t[:], in_=partials[:]).then_inc(s_out, 16)

    # Bacc defers register allocation + event-semaphore splitting to
    # compile(); the pjrt exec path serializes without calling it.
    nc.compile()
    return nc


def get_nc():
    if "nc" not in _NC_CACHE:
        _NC_CACHE["nc"] = _build_bass()
    return _NC_CACHE["nc"]


def kernel(x, labels, centers, _run_kwargs=None):
    x = np.ascontiguousarray(x, dtype=np.float32)
    labels = np.ascontiguousarray(labels).astype(np.int32)
    centers = np.ascontiguousarray(centers, dtype=np.float32)

    nc = get_nc()
    in_maps = [
        {
            "x": x[c * BPC:(c + 1) * BPC],
            "labels": labels[c * BPC:(c + 1) * BPC],
            "centers": centers,
        }
        for c in range(N_CORES)
    ]
    kwargs = _run_kwargs or {}
    out = run_bass_kernel_spmd(nc, in_maps, core_ids=list(range(N_CORES)),
                               **kwargs)
    # reduce the 8 per-core [128, 2] partial-sum tiles on the host
    total = np.float64(0.0)
    for r in out.results:
        total += np.asarray(r["out"], dtype=np.float64).sum()
    if kwargs:
        kernel.last_run = out
    return np.asarray(total / BATCH, dtype=np.float32)
